# revision 11
# baseline (speedup 1.0000x reference)
"""2-layer GAT (PyG GATConv semantics) on 8 trn2 NeuronCores via Bass/Tile.

Sharding: nodes are sharded contiguously across cores (dst-sharding of
edges); each core processes all incoming edges of its node shard.
Per-core node indices are ROTATED (core k's row r = node (k*NSH + r) % N)
so that every core runs the IDENTICAL program (SPMD) and all
core-dependence lives in the input data.

Device pipeline per core:
  phase1: h1 = x @ W1.T and attention half-scores via one matmul per
          128-node tile (bf16); writes a gather table row1[100000, 512B]
          = [h1 bf16 128 | al_src1 fp32 x4] and a shard-local dst table
          dtab1[12500, 256B] = [al_dst1 fp32 x4].
  pass A (layer-1 edge phase): edges in (src-subtable, dst-block) order,
          chunks of 128 edges on partitions; dma_gather of row1 rows by
          src (512B) + dtab1 rows by dst (256B); scores
          ex = exp(leakyrelu(al_src+al_dst)); one-hot S (is_equal vs
          iota matrix) per chunk; PE matmul S^T @ [h*ex | ex] accumulated
          per dst-block; block tails: softmax-normalize, bias, relu,
          then h2 = o1 @ W2.T (+ layer-2 half-scores) -> row2 shard +
          dtab2.
  AllGather row2 shards -> row2full [100000, 256B].
  pass B (layer-2 edge phase): same structure, 16 channels, 1 head.
Output: o2 shard [12500, 16] fp32 per core, concatenated on host.
"""

import os

os.environ.setdefault("NEURON_RT_RESET_CORES", "1")

import numpy as np
import ml_dtypes
from contextlib import ExitStack

import concourse.bass as bass
import concourse.tile as tile
from concourse import bacc, mybir
from concourse.bass_utils import run_bass_kernel_spmd
from concourse.masks import make_identity

bf16 = ml_dtypes.bfloat16
f32 = np.float32
dt = mybir.dt

P = 128
NEG = 0.2

LAST_HW_EXEC_NS = None


class Cfg:
    def __init__(self, N, F, HID, HEADS, CLS, ncores, sub, sch):
        self.N, self.F, self.HID, self.HEADS, self.CLS = N, F, HID, HEADS, CLS
        self.ncores = ncores
        self.NSH = N // ncores           # nodes per core
        self.SUB = sub                   # subtable rows (int16 range)
        self.NSUB = (N + sub - 1) // sub
        self.SCH = sch                   # chunks per supertile
        self.NBLK = (self.NSH + P - 1) // P
        self.HC = HEADS * HID            # 128
        assert self.F == P and self.HC == P
        # row1: [h1 bf16 HC | al_src fp32 HEADS] padded to 256 bf16 units
        self.R1W = 256                   # bf16 units (512B)
        self.R2W = 128                   # row2 row width bf16 (256B)
        self.DTW = 128                   # dst-table row width bf16 (256B)


FULL = Cfg(N=100_000, F=128, HID=32, HEADS=4, CLS=16, ncores=8, sub=25_000, sch=48)


# ----------------------------------------------------------------- host prep

def _wrap_idx(idx_chunks):
    """[C, 128] int -> wrapped int16 [C, 128, 8] (16-partition wrap, x8)."""
    C = idx_chunks.shape[0]
    w = idx_chunks.reshape(C, 8, 16).transpose(0, 2, 1)      # [C, 16, 8]
    return np.tile(w, (1, 8, 1)).astype(np.int16)            # [C, 128, 8]


def _prep(cfg, x, edge_index):
    """Build per-core inputs + the (uniform) chunk schedule."""
    N, NSH, SUB, SCH, NBLK = cfg.N, cfg.NSH, cfg.SUB, cfg.SCH, cfg.NBLK
    K = cfg.ncores
    NSUB = cfg.NSUB

    loop = np.arange(N, dtype=np.int64)
    src = np.concatenate([edge_index[0].astype(np.int64), loop])
    dst = np.concatenate([edge_index[1].astype(np.int64), loop])
    core = dst // NSH

    # per-core edge fields
    per = []
    cnt1 = np.zeros((K, NSUB, NBLK), np.int64)
    cnt2 = np.zeros((K, NSUB, NBLK), np.int64)
    for k in range(K):
        m = core == k
        s, d = src[m], dst[m]
        srcrot = (s - k * NSH) % N
        dstloc = d - k * NSH
        b = dstloc // P
        c = dstloc - b * P
        t1 = srcrot // SUB
        t2 = s // SUB
        cnt1[k] = np.bincount(t1 * NBLK + b, minlength=NSUB * NBLK).reshape(NSUB, NBLK)
        cnt2[k] = np.bincount(t2 * NBLK + b, minlength=NSUB * NBLK).reshape(NSUB, NBLK)
        per.append(dict(srcrot=srcrot, dstloc=dstloc, b=b, c=c, t1=t1, t2=t2))

    G1 = -(-cnt1.max(axis=0) // P)       # [NSUB, NBLK] chunks, uniform
    G2 = -(-cnt2.max(axis=0) // P)

    def build_stream(G):
        chunks = []                       # (t, b) or None
        for t in range(NSUB):
            for b in range(NBLK):
                chunks += [(t, b)] * int(G[t, b])
        npad = (-len(chunks)) % SCH
        chunks += [None] * npad
        return chunks

    chunksA = build_stream(G1)
    chunksB = build_stream(G2)

    def build_core_arrays(k, G, tkey, locfield):
        """Per-core edge arrays in stream order, padded per (t,b) group.

        Returns srcloc [C,128] int16, dstloc [C,128] int16, c [C,128] bf16.
        """
        p = per[k]
        tv = p[tkey]
        C = int(G.sum())
        key = (tv * NBLK + p["b"]).astype(np.int64)
        order = np.argsort(key, kind="stable")
        sl = np.zeros((C, P), np.int16)
        dl = np.zeros((C, P), np.int16)
        cv = np.full((C, P), P, np.float32)       # pad c = 128 -> zero one-hot
        # group offsets in the chunk stream
        goff = {}
        off = 0
        for t in range(NSUB):
            for b in range(NBLK):
                if G[t, b]:
                    goff[(t, b)] = off
                    off += int(G[t, b])
        counts = np.bincount(key, minlength=NSUB * NBLK)
        starts = np.concatenate([[0], np.cumsum(counts)])
        srcl = (p[locfield] % SUB).astype(np.int16)[order]
        dstl = p["dstloc"].astype(np.int16)[order]
        cvs = p["c"].astype(np.float32)[order]
        for t in range(NSUB):
            for b in range(NBLK):
                n = counts[t * NBLK + b]
                if n == 0:
                    continue
                e0 = starts[t * NBLK + b]
                g0 = goff[(t, b)]
                ncap = int(G[t, b]) * P
                flat_s = np.zeros(ncap, np.int16)
                flat_d = np.zeros(ncap, np.int16)
                flat_c = np.full(ncap, P, np.float32)
                flat_s[:n] = srcl[e0:e0 + n]
                flat_d[:n] = dstl[e0:e0 + n]
                flat_c[:n] = cvs[e0:e0 + n]
                sl[g0:g0 + ncap // P] = flat_s.reshape(-1, P)
                dl[g0:g0 + ncap // P] = flat_d.reshape(-1, P)
                cv[g0:g0 + ncap // P] = flat_c.reshape(-1, P)
        return sl, dl, cv.astype(bf16)

    def build_blob(sl, dl, cv, nchunks_padded):
        C = sl.shape[0]
        Cp = nchunks_padded
        if Cp > C:
            sl = np.concatenate([sl, np.zeros((Cp - C, P), np.int16)])
            dl = np.concatenate([dl, np.zeros((Cp - C, P), np.int16)])
            cv = np.concatenate([cv, np.full((Cp - C, P), P, np.float32).astype(bf16)])
        ws = _wrap_idx(sl)                       # [Cp, 128, 8] i16
        wd = _wrap_idx(dl)
        NST = Cp // SCH
        blob = np.zeros((NST, P, 2 * SCH * 16 + SCH * 2), np.uint8)
        ws = ws.reshape(NST, SCH, P, 8).transpose(0, 2, 1, 3)    # [NST,128,SCH,8]
        wd = wd.reshape(NST, SCH, P, 8).transpose(0, 2, 1, 3)
        cvr = cv.reshape(NST, SCH, P).transpose(0, 2, 1)         # [NST,128,SCH]
        blob[:, :, 0:SCH * 16] = ws.reshape(NST, P, SCH * 8).view(np.uint8)
        blob[:, :, SCH * 16:2 * SCH * 16] = wd.reshape(NST, P, SCH * 8).view(np.uint8)
        blob[:, :, 2 * SCH * 16:] = np.ascontiguousarray(cvr).view(np.uint8)
        return blob

    blobsA, blobsB = [], []
    CpA = len(chunksA)
    CpB = len(chunksB)
    for k in range(K):
        sl, dl, cv = build_core_arrays(k, G1, "t1", "srcrot")
        blobsA.append(build_blob(sl, dl, cv, CpA))
        sl, dl, cv = build_core_arrays(k, G2, "t2", "srcrot")
        # pass B src idx uses UNROTATED node order
        p = per[k]
        order = np.argsort((p["t2"] * NBLK + p["b"]).astype(np.int64), kind="stable")
        # rebuild srcloc2 with same padding layout
        sl2 = np.zeros_like(sl)
        counts = np.bincount((p["t2"] * NBLK + p["b"]).astype(np.int64),
                             minlength=NSUB * NBLK)
        starts = np.concatenate([[0], np.cumsum(counts)])
        srcl2 = (src[core == k] % SUB).astype(np.int16)[order]
        off = 0
        for t in range(NSUB):
            for b in range(NBLK):
                gg = int(G2[t, b])
                if gg == 0:
                    continue
                n = counts[t * NBLK + b]
                e0 = starts[t * NBLK + b]
                ncap = gg * P
                flat = np.zeros(ncap, np.int16)
                flat[:n] = srcl2[e0:e0 + n]
                sl2[off:off + gg] = flat.reshape(-1, P)
                off += gg
        blobsB.append(build_blob(sl2, dl, cv, CpB))

    return dict(G1=G1, G2=G2, chunksA=chunksA, chunksB=chunksB,
                blobsA=blobsA, blobsB=blobsB)


def _host_tables(cfg, x, W1, a_src1, a_dst1, b1, W2, a_src2, a_dst2, b2):
    """Weight-derived constant inputs (shared across cores except xT)."""
    F, HC, HEADS, HID, CLS = cfg.F, cfg.HC, cfg.HEADS, cfg.HID, cfg.CLS
    # B1src[cin, h] = sum_c W1[h*HID+c, cin] * a_src1[h, c]
    W1r = W1.reshape(HEADS, HID, F)
    B1s = np.einsum("hcf,hc->fh", W1r, a_src1).astype(f32)
    B1d = np.einsum("hcf,hc->fh", W1r, a_dst1).astype(f32)
    wb1 = np.concatenate([W1.T, B1s, B1d], axis=1).astype(bf16)   # [F, HC+2H]
    # W2A2: [HC, CLS | B2s | B2d]
    B2s = (W2.T @ a_src2[0]).astype(f32)[:, None]
    B2d = (W2.T @ a_dst2[0]).astype(f32)[:, None]
    w2a2 = np.concatenate([W2.T, B2s, B2d], axis=1).astype(bf16)  # [HC, CLS+2]
    b1t = np.tile(b1.astype(f32)[None, :], (P, 1)).astype(bf16)
    b2t = np.tile(b2.astype(f32)[None, :], (P, 1)).astype(f32)
    jc = np.tile(np.arange(P, dtype=np.float32)[None, :], (P, 1)).astype(bf16)
    return wb1, w2a2, b1t, b2t, jc


# ------------------------------------------------------------- device program

def _build_program(cfg, sched, use_b1=True, use_b2=True):
    N, NSH, SUB, SCH, NBLK, NSUB = (cfg.N, cfg.NSH, cfg.SUB, cfg.SCH,
                                    cfg.NBLK, cfg.NSUB)
    HEADS, CLS, HC = cfg.HEADS, cfg.CLS, cfg.HC
    G1, G2 = sched["G1"], sched["G2"]
    chunksA, chunksB = sched["chunksA"], sched["chunksB"]
    NSTA = len(chunksA) // SCH
    NSTB = len(chunksB) // SCH
    BLOBW = 2 * SCH * 16 + SCH * 2

    nc = bacc.Bacc("TRN2", target_bir_lowering=False, debug=False,
                   enable_asserts=True, num_devices=cfg.ncores)

    xT = nc.dram_tensor("xT", [P, N], dt.bfloat16, kind="ExternalInput").ap()
    wb1_d = nc.dram_tensor("wb1", [P, HC + 2 * HEADS], dt.bfloat16,
                           kind="ExternalInput").ap()
    w2a2_d = nc.dram_tensor("w2a2", [P, CLS + 2], dt.bfloat16,
                            kind="ExternalInput").ap()
    b1t_d = nc.dram_tensor("b1t", [P, HC], dt.bfloat16, kind="ExternalInput").ap()
    b2t_d = nc.dram_tensor("b2t", [P, CLS], dt.float32, kind="ExternalInput").ap()
    jc_d = nc.dram_tensor("jc", [P, P], dt.bfloat16, kind="ExternalInput").ap()
    blobA_d = nc.dram_tensor("blobA", [NSTA, P, BLOBW], dt.uint8,
                             kind="ExternalInput").ap()
    blobB_d = nc.dram_tensor("blobB", [NSTB, P, BLOBW], dt.uint8,
                             kind="ExternalInput").ap()

    o2_d = nc.dram_tensor("o2", [NSH, CLS], dt.float32, kind="ExternalOutput").ap()

    row1 = nc.dram_tensor("row1", [N, cfg.R1W], dt.bfloat16).ap()
    dtab1 = nc.dram_tensor("dtab1", [NSH, cfg.DTW], dt.bfloat16).ap()
    row2sh = nc.dram_tensor("row2sh", [NSH, cfg.R2W], dt.bfloat16).ap()
    row2full = nc.dram_tensor("row2full", [N, cfg.R2W], dt.bfloat16,
                              addr_space="Shared").ap()
    dtab2 = nc.dram_tensor("dtab2", [NSH, cfg.DTW], dt.bfloat16).ap()

    AluOp = mybir.AluOpType
    Act = mybir.ActivationFunctionType

    with tile.TileContext(nc) as tc, ExitStack() as ctx:
        cpool = ctx.enter_context(tc.tile_pool(name="consts", bufs=1))
        wb1_s = cpool.tile([P, HC + 2 * HEADS], dt.bfloat16)
        nc.sync.dma_start(wb1_s[:], wb1_d[:])
        w2a2_s = cpool.tile([P, CLS + 2], dt.bfloat16)
        nc.sync.dma_start(w2a2_s[:], w2a2_d[:])
        b1t_s = cpool.tile([P, HC], dt.bfloat16)
        nc.sync.dma_start(b1t_s[:], b1t_d[:])
        b2t_s = cpool.tile([P, CLS], dt.float32)
        nc.sync.dma_start(b2t_s[:], b2t_d[:])
        jc_s = cpool.tile([P, P], dt.bfloat16)
        nc.sync.dma_start(jc_s[:], jc_d[:])
        ident = cpool.tile([P, P], dt.bfloat16)
        make_identity(nc, ident[:])

        accA = cpool.tile([P, NBLK, HC + HEADS], dt.float32)
        accB = cpool.tile([P, NBLK, CLS + 1], dt.float32)

        # ---------------- phase 1: h1 + attention half-scores -> row1, dtab1
        GT = 32  # node-tiles per group
        ntile_full = N // P              # full 128-node tiles
        rem = N - ntile_full * P
        with ExitStack() as p1:
            pool = p1.enter_context(tc.tile_pool(name="p1", bufs=3))
            ps = p1.enter_context(tc.tile_pool(name="p1ps", bufs=4, space="PSUM"))
            tiles_done = 0
            while tiles_done < ntile_full:
                nt = min(GT, ntile_full - tiles_done)
                n0 = tiles_done * P
                xg = pool.tile([P, GT * P], dt.bfloat16, tag="xg")
                nc.sync.dma_start(xg[:, 0:nt * P], xT[:, n0:n0 + nt * P])
                rowbuf = pool.tile([P, GT, HC + 2 * HEADS], dt.bfloat16, tag="rowbuf")
                need_d = n0 < NSH
                if need_d:
                    dbuf = pool.tile([P, GT, 8], dt.bfloat16, tag="dbuf")
                for j in range(nt):
                    ps1 = ps.tile([P, HC + 2 * HEADS], dt.float32, space="PSUM")
                    nc.tensor.matmul(ps1[:], lhsT=xg[:, j * P:(j + 1) * P],
                                     rhs=wb1_s[:], start=True, stop=True)
                    nc.scalar.activation(rowbuf[:, j, 0:HC], ps1[:, 0:HC], Act.Copy)
                    nc.vector.tensor_copy(
                        rowbuf[:, j, HC:HC + 2 * HEADS].bitcast(dt.float32),
                        ps1[:, HC:HC + HEADS])
                    if need_d and n0 + j * P < NSH:
                        nrow = min(P, NSH - (n0 + j * P))
                        nc.vector.tensor_copy(
                            dbuf[0:nrow, j, 0:8].bitcast(dt.float32),
                            ps1[0:nrow, HC + HEADS:HC + 2 * HEADS])
                # write row1 rows [n0 : n0+nt*128), payload cols [0:136]
                outap = row1[n0:n0 + nt * P, 0:HC + 2 * HEADS].rearrange(
                    "(j p) c -> p j c", p=P)
                nc.sync.dma_start(outap, rowbuf[:, 0:nt, :])
                if need_d:
                    ncov = min(nt, (NSH - n0) // P)   # fully covered tiles
                    if ncov > 0:
                        outap = dtab1[n0:n0 + ncov * P, 0:8].rearrange(
                            "(j p) c -> p j c", p=P)
                        nc.sync.dma_start(outap, dbuf[:, 0:ncov, :])
                    if ncov < nt and n0 + ncov * P < NSH:
                        r0 = n0 + ncov * P
                        nrow = NSH - r0
                        nc.sync.dma_start(dtab1[r0:r0 + nrow, 0:8],
                                          dbuf[0:nrow, ncov, :])
                tiles_done += nt
            if rem:
                n0 = ntile_full * P
                xg = pool.tile([P, GT * P], dt.bfloat16, tag="xg")
                nc.sync.dma_start(xg[:, 0:rem], xT[:, n0:n0 + rem])
                rowbuf = pool.tile([P, GT, HC + 2 * HEADS], dt.bfloat16, tag="rowbuf")
                ps1 = ps.tile([P, HC + 2 * HEADS], dt.float32, space="PSUM")
                nc.tensor.matmul(ps1[0:rem, :], lhsT=xg[:, 0:rem], rhs=wb1_s[:],
                                 start=True, stop=True)
                nc.scalar.activation(rowbuf[0:rem, 0, 0:HC], ps1[0:rem, 0:HC],
                                     Act.Copy)
                nc.vector.tensor_copy(
                    rowbuf[0:rem, 0, HC:HC + 2 * HEADS].bitcast(dt.float32),
                    ps1[0:rem, HC:HC + HEADS])
                nc.sync.dma_start(row1[n0:n0 + rem, 0:HC + 2 * HEADS],
                                  rowbuf[0:rem, 0, :])

        # ---------------- generic edge pass -------------------------------
        def edge_pass(chunks, G, blob_d, NST, srctab_views, dsttab, rowW,
                      nheads, msgW, do_tail):
            """msgW: message channels (HC or CLS); rhs width = msgW + nheads."""
            # per-(t,b) group extents
            first_chunk = {}
            last_chunk = {}
            for i, tb in enumerate(chunks):
                if tb is None:
                    continue
                if tb not in first_chunk:
                    first_chunk[tb] = i
                last_chunk[tb] = i
            tmin = {}
            tmax = {}
            for (t, b) in first_chunk:
                tmin.setdefault(b, t)
                tmin[b] = min(tmin[b], t)
                tmax.setdefault(b, t)
                tmax[b] = max(tmax[b], t)

            with ExitStack() as es:
                pool = es.enter_context(tc.tile_pool(name="ep", bufs=2))
                pspool = es.enter_context(
                    tc.tile_pool(name="epps", bufs=4, space="PSUM"))
                tpool = es.enter_context(tc.tile_pool(name="tail", bufs=2))
                tps = es.enter_context(
                    tc.tile_pool(name="tailps", bufs=2, space="PSUM"))
                cur_ps = None       # psum tile of open (t,b) group
                for s in range(NST):
                    cks = chunks[s * SCH:(s + 1) * SCH]
                    blob = pool.tile([P, BLOBW], dt.uint8, tag="blob")
                    nc.sync.dma_start(blob[:], blob_d[s])
                    gsrc = pool.tile([P, SCH, rowW], dt.bfloat16, tag="gsrc")
                    # src gather segments (runs of same t)
                    g0 = 0
                    while g0 < SCH:
                        tb = cks[g0]
                        t = tb[0] if tb is not None else NSUB - 1
                        g1 = g0 + 1
                        while g1 < SCH:
                            tb1 = cks[g1]
                            t1 = tb1[0] if tb1 is not None else NSUB - 1
                            if t1 != t:
                                break
                            g1 += 1
                        ni = (g1 - g0) * P
                        nc.gpsimd.dma_gather(
                            out_ap=gsrc[:, g0:g1, :],
                            in_ap=srctab_views[t],
                            idxs_ap=blob[:, g0 * 16:g0 * 16 + (g1 - g0) * 16]
                                    .bitcast(dt.int16),
                            num_idxs=ni, num_idxs_reg=ni, elem_size=rowW,
                            single_packet=False)
                        g0 = g1
                    gdst = pool.tile([P, SCH, cfg.DTW], dt.bfloat16, tag="gdst")
                    nc.gpsimd.dma_gather(
                        out_ap=gdst[:], in_ap=dsttab,
                        idxs_ap=blob[:, SCH * 16:2 * SCH * 16].bitcast(dt.int16),
                        num_idxs=SCH * P, num_idxs_reg=SCH * P,
                        elem_size=cfg.DTW, single_packet=False)
                    # scores
                    als = gsrc[:, :, msgW:msgW + 2 * nheads].bitcast(dt.float32)
                    ald = gdst[:, :, 0:2 * nheads].bitcast(dt.float32)
                    u = pool.tile([P, SCH, nheads], dt.float32, tag="u")
                    nc.vector.tensor_tensor(out=u[:], in0=als, in1=ald,
                                            op=AluOp.add)
                    u2 = pool.tile([P, SCH, nheads], dt.float32, tag="u2")
                    nc.vector.tensor_scalar_mul(u2[:], u[:], NEG)
                    nc.vector.tensor_tensor(out=u[:], in0=u[:], in1=u2[:],
                                            op=AluOp.max)
                    ex = pool.tile([P, SCH, nheads], dt.bfloat16, tag="ex")
                    nc.scalar.activation(ex[:], u[:], Act.Exp)
                    # one-hot S
                    S = pool.tile([P, SCH, P], dt.bfloat16, tag="S")
                    cvals = blob[:, 2 * SCH * 16:].bitcast(dt.bfloat16)  # [P,SCH]
                    nc.vector.tensor_tensor(
                        out=S[:],
                        in0=jc_s[:].unsqueeze(1).to_broadcast([P, SCH, P]),
                        in1=cvals.unsqueeze(2).to_broadcast([P, SCH, P]),
                        op=AluOp.is_equal)
                    # mx = [h * ex | ex]
                    mx = pool.tile([P, SCH, msgW + nheads], dt.bfloat16, tag="mx")
                    ch_per_head = msgW // nheads
                    nc.vector.tensor_tensor(
                        out=mx[:, :, 0:msgW].rearrange(
                            "p s (h c) -> p s h c", c=ch_per_head),
                        in0=gsrc[:, :, 0:msgW].rearrange(
                            "p s (h c) -> p s h c", c=ch_per_head),
                        in1=ex[:].unsqueeze(3).to_broadcast(
                            [P, SCH, nheads, ch_per_head]),
                        op=AluOp.mult)
                    nc.vector.tensor_copy(mx[:, :, msgW:msgW + nheads], ex[:])
                    # matmuls per chunk
                    for g, tb in enumerate(cks):
                        if tb is None:
                            continue
                        t, b = tb
                        gi = s * SCH + g
                        if gi == first_chunk[tb]:
                            cur_ps = pspool.tile([P, msgW + nheads], dt.float32,
                                                 space="PSUM", tag="acc")
                        nc.tensor.matmul(cur_ps[:], lhsT=S[:, g, :],
                                         rhs=mx[:, g, :],
                                         start=(gi == first_chunk[tb]),
                                         stop=(gi == last_chunk[tb]))
                        if gi == last_chunk[tb]:
                            accT = accA if do_tail else accB
                            accv = accT[:, b, :]
                            if t == tmin[b]:
                                nc.vector.tensor_copy(accv, cur_ps[:])
                            else:
                                nc.vector.tensor_tensor(out=accv, in0=accv,
                                                        in1=cur_ps[:],
                                                        op=AluOp.add)
                            if t == tmax[b]:
                                if do_tail:
                                    tail_A(b, tpool, tps)
                                else:
                                    tail_B(b, tpool)

        def tail_A(b, tpool, tps):
            nb = min(P, NSH - b * P)
            acc = accA[:, b, :]
            zinv = tpool.tile([P, HEADS], dt.float32, tag="zinv")
            nc.vector.reciprocal(zinv[:], acc[:, HC:HC + HEADS])
            o1b = tpool.tile([P, HC], dt.bfloat16, tag="o1b")
            for h in range(HEADS):
                HL = HC // HEADS
                nc.vector.tensor_scalar(
                    out=o1b[:, h * HL:(h + 1) * HL],
                    in0=acc[:, h * HL:(h + 1) * HL],
                    scalar1=zinv[:, h:h + 1], scalar2=None, op0=AluOp.mult)
            if use_b1:
                nc.vector.tensor_tensor(out=o1b[:], in0=o1b[:], in1=b1t_s[:],
                                        op=AluOp.add)
            nc.vector.tensor_scalar(out=o1b[:], in0=o1b[:], scalar1=0.0,
                                    scalar2=None, op0=AluOp.max)
            psT = tps.tile([P, P], dt.bfloat16, space="PSUM", tag="psT")
            nc.tensor.transpose(out=psT[:], in_=o1b[:], identity=ident[:])
            o1T = tpool.tile([P, P], dt.bfloat16, tag="o1T")
            nc.vector.tensor_copy(o1T[:], psT[:])
            ps2 = tps.tile([P, CLS + 2], dt.float32, space="PSUM", tag="ps2")
            nc.tensor.matmul(ps2[:], lhsT=o1T[:], rhs=w2a2_s[:],
                             start=True, stop=True)
            r2 = tpool.tile([P, CLS + 2], dt.bfloat16, tag="r2")
            nc.vector.tensor_copy(r2[:, 0:CLS], ps2[:, 0:CLS])
            nc.vector.tensor_copy(r2[:, CLS:CLS + 2].bitcast(dt.float32),
                                  ps2[:, CLS:CLS + 1])
            d2 = tpool.tile([P, 2], dt.bfloat16, tag="d2")
            nc.vector.tensor_copy(d2[:].bitcast(dt.float32),
                                  ps2[:, CLS + 1:CLS + 2])
            nc.sync.dma_start(row2sh[b * P:b * P + nb, 0:CLS + 2], r2[0:nb, :])
            nc.sync.dma_start(dtab2[b * P:b * P + nb, 0:2], d2[0:nb, :])

        def tail_B(b, tpool):
            nb = min(P, NSH - b * P)
            acc = accB[:, b, :]
            zinv = tpool.tile([P, 1], dt.float32, tag="zinv2")
            nc.vector.reciprocal(zinv[:], acc[:, CLS:CLS + 1])
            o2t = tpool.tile([P, CLS], dt.float32, tag="o2t")
            nc.vector.tensor_scalar(out=o2t[:], in0=acc[:, 0:CLS],
                                    scalar1=zinv[:, 0:1], scalar2=None,
                                    op0=AluOp.mult)
            if use_b2:
                nc.vector.tensor_tensor(out=o2t[:], in0=o2t[:], in1=b2t_s[:],
                                        op=AluOp.add)
            nc.sync.dma_start(o2_d[b * P:b * P + nb, :], o2t[0:nb, :])

        # pass A
        r1views = [row1[t * SUB:min((t + 1) * SUB, N), :] for t in range(NSUB)]
        edge_pass(chunksA, G1, blobA_d, NSTA, r1views, dtab1[:], cfg.R1W,
                  HEADS, HC, do_tail=True)

        # collective: row2 shards -> full
        nc.gpsimd.collective_compute(
            "AllGather", AluOp.bypass,
            replica_groups=[list(range(cfg.ncores))],
            ins=[row2sh[:]], outs=[row2full[:]])

        # pass B
        r2views = [row2full[t * SUB:min((t + 1) * SUB, N), :] for t in range(NSUB)]
        edge_pass(chunksB, G2, blobB_d, NSTB, r2views, dtab2[:], cfg.R2W,
                  1, CLS, do_tail=False)

    nc.compile()
    return nc


# ------------------------------------------------------------------- entry

def _run_pjrt_timed(nc, in_maps, n_cores, time_reps=12):
    """run_bass_via_pjrt with inputs pre-staged on device and the execute
    timed separately (this environment has no NTFF profiling hook)."""
    import jax
    from jax.sharding import Mesh, PartitionSpec
    from jax.experimental.shard_map import shard_map
    import time as _time
    from concourse import bass2jax, mybir as _mb

    bass2jax.install_neuronx_cc_hook()
    partition_name = (nc.partition_id_tensor.name
                      if nc.partition_id_tensor else None)
    in_names, out_names, out_avals, zero_outs = [], [], [], []
    for alloc in nc.m.functions[0].allocations:
        if not isinstance(alloc, _mb.MemoryLocationSet):
            continue
        name = alloc.memorylocations[0].name
        if alloc.kind == "ExternalInput":
            if name != partition_name:
                in_names.append(name)
        elif alloc.kind == "ExternalOutput":
            shape = tuple(alloc.tensor_shape)
            dtype = _mb.dt.np(alloc.dtype)
            out_names.append(name)
            out_avals.append(jax.core.ShapedArray(shape, dtype))
            zero_outs.append(np.zeros(shape, dtype))
    n_params = len(in_names)
    n_outs = len(out_avals)
    in_names_all = list(in_names) + list(out_names)
    if partition_name is not None:
        in_names_all.append(partition_name)

    def _body(*args):
        operands = list(args)
        if partition_name is not None:
            operands.append(bass2jax.partition_id_tensor())
        outs = bass2jax._bass_exec_p.bind(
            *operands, out_avals=tuple(out_avals), in_names=tuple(in_names_all),
            out_names=tuple(out_names), lowering_input_output_aliases=(),
            sim_require_finite=True, sim_require_nnan=True, nc=nc)
        return tuple(outs)

    devices = jax.devices()[:n_cores]
    mesh = Mesh(np.asarray(devices), ("core",))
    in_specs = (PartitionSpec("core"),) * (n_params + n_outs)
    out_specs = (PartitionSpec("core"),) * n_outs
    donate = tuple(range(n_params, n_params + n_outs))
    sharded = jax.jit(
        shard_map(_body, mesh=mesh, in_specs=in_specs, out_specs=out_specs,
                  check_rep=False),
        donate_argnums=donate, keep_unused=True)

    concat_in = [
        np.concatenate([np.asarray(in_maps[c][nm]) for c in range(n_cores)], 0)
        for nm in in_names]
    concat_zeros = [np.zeros((n_cores * z.shape[0], *z.shape[1:]), z.dtype)
                    for z in zero_outs]
    sh = jax.sharding.NamedSharding(mesh, PartitionSpec("core"))
    dev_in = [jax.device_put(a, sh) for a in concat_in]
    for a in dev_in:
        a.block_until_ready()

    # warmup (includes jit + NEFF compile)
    out_arrs = sharded(*dev_in, *[jax.device_put(z, sh) for z in concat_zeros])
    for o in out_arrs:
        o.block_until_ready()
    times = []
    for _ in range(time_reps):
        zs = [jax.device_put(z, sh) for z in concat_zeros]
        for z in zs:
            z.block_until_ready()
        t0 = _time.perf_counter()
        out_arrs = sharded(*dev_in, *zs)
        for o in out_arrs:
            o.block_until_ready()
        times.append(_time.perf_counter() - t0)
    exec_ns = int(min(times) * 1e9)
    results = [
        {nm: np.asarray(out_arrs[i]).reshape(n_cores, *out_avals[i].shape)[c]
         for i, nm in enumerate(out_names)}
        for c in range(n_cores)]
    return results, exec_ns


def _run(cfg, inputs, trace=False):
    global LAST_HW_EXEC_NS
    x = np.asarray(inputs["x"], f32)
    ei = np.asarray(inputs["edge_index"])
    sched = _prep(cfg, x, ei)
    wb1, w2a2, b1t, b2t, jc = _host_tables(
        cfg, x,
        np.asarray(inputs["W1"], f32), np.asarray(inputs["a_src1"], f32),
        np.asarray(inputs["a_dst1"], f32), np.asarray(inputs["b1"], f32),
        np.asarray(inputs["W2"], f32), np.asarray(inputs["a_src2"], f32),
        np.asarray(inputs["a_dst2"], f32), np.asarray(inputs["b2"], f32))
    nc = _build_program(cfg, sched,
                        use_b1=bool(np.any(np.asarray(inputs["b1"]))),
                        use_b2=bool(np.any(np.asarray(inputs["b2"]))))

    xTb = np.ascontiguousarray(x.T).astype(bf16)       # [F, N]
    in_maps = []
    for k in range(cfg.ncores):
        roll = np.roll(xTb, -k * cfg.NSH, axis=1) if k else xTb
        in_maps.append(dict(
            xT=np.ascontiguousarray(roll), wb1=wb1, w2a2=w2a2, b1t=b1t,
            b2t=b2t, jc=jc, blobA=sched["blobsA"][k], blobB=sched["blobsB"][k]))
    results, exec_ns = _run_pjrt_timed(nc, in_maps, cfg.ncores)
    LAST_HW_EXEC_NS = exec_ns
    out = np.concatenate([results[k]["o2"] for k in range(cfg.ncores)], 0)
    return out.astype(f32)


def kernel(x, edge_index, W1, a_src1, a_dst1, b1, W2, a_src2, a_dst2, b2):
    return _run(FULL, dict(x=x, edge_index=edge_index, W1=W1, a_src1=a_src1,
                           a_dst1=a_dst1, b1=b1, W2=W2, a_src2=a_src2,
                           a_dst2=a_dst2, b2=b2),
                trace=os.environ.get("GAT_TRACE", "0") == "1")
